# revision 9
# baseline (speedup 1.0000x reference)
"""4-layer GAT on 8 Trainium2 NeuronCores (Bass/Tile, SPMD).

Sharding: nodes are partitioned across the 8 cores (graph parallel); edges are
sharded by destination node so segment-softmax and the weighted scatter-add
stay core-local. Per layer, each core projects its own nodes (GEMM), the
per-node rows [h | alpha_src] are AllGathered into a per-core DRAM gather
table, and the edge phase gathers source rows by index (SWDGE dma_gather) in a
degree-sorted, per-destination-padded layout: gather slot (d, j) holds the
j-th in-edge of block-local destination d, so partition d of the gathered tile
is destination d. Segment sums are then computed with identity-lhsT matmuls
accumulating over slots in PSUM. Softmax is computed without the segment-max
shift (mathematically identical after normalization); pad slots get a -30000
logit bias so they contribute exactly 0.
"""
import math
import numpy as np

import concourse.bass as bass
import concourse.bacc as bacc
import concourse.mybir as mybir
import concourse.tile as tile
from concourse import bass_utils

FP16 = mybir.dt.float16
FP32 = mybir.dt.float32
I16 = mybir.dt.int16

N_CORES = 8
JC = 8          # gather-chunk size in slots
BN_EPS = 1e-5
NEG_SLOPE = 0.2


# ----------------------------------------------------------------- host prep

def _prep_graph(n, edge_index, npc, npad):
    """Edge structure -> per-core degree-sorted blocks + unified slot counts.

    Returns dict with: perm (per-core list of global node ids in processing
    order), g2v, D (unified per-block slot counts), per-core idx images and
    masks.
    """
    e = edge_index
    src = np.concatenate([e[0].astype(np.int64), np.arange(n, dtype=np.int64)])
    dst = np.concatenate([e[1].astype(np.int64), np.arange(n, dtype=np.int64)])
    deg = np.bincount(dst, minlength=n)

    order = np.argsort(dst, kind="stable")
    src_sorted = src[order]
    # start offset of each node's edge run in src_sorted
    starts = np.zeros(n + 1, np.int64)
    np.cumsum(deg, out=starts[1:])

    nblk = npad // 128
    perms = []          # per core: global ids, processing order (len npc)
    for c in range(N_CORES):
        g0 = c * npc
        local = np.arange(g0, g0 + npc)
        p = local[np.argsort(-deg[local], kind="stable")]
        perms.append(p)

    g2v = np.full(n, -1, np.int64)
    for c in range(N_CORES):
        g2v[perms[c]] = c * npad + np.arange(npc)

    # unified per-block slot counts
    D = np.zeros(nblk, np.int64)
    for c in range(N_CORES):
        dsort = deg[perms[c]]
        for b in range(nblk):
            blk = dsort[b * 128:(b + 1) * 128]
            if len(blk):
                D[b] = max(D[b], int(blk.max()))
    D = np.maximum(D, 1)

    # chunk layout (same for all cores)
    chunks = []         # (block, j0, jc, idx_col_off, mask_col_off)
    icol = 0
    mcol = 0
    for b in range(nblk):
        j0 = 0
        while j0 < D[b]:
            jc = min(JC, int(D[b]) - j0)
            chunks.append((b, j0, jc, icol, mcol + j0))
            icol += (128 * jc) // 16
            j0 += jc
        mcol += int(D[b])
    icols_total = icol
    sumd = mcol

    idx_imgs = np.zeros((N_CORES, 128, icols_total), np.int16)
    masks = np.zeros((N_CORES, 128, sumd), np.float16)
    for c in range(N_CORES):
        p = perms[c]
        mdcol = 0
        for b in range(nblk):
            for d in range(128):
                pos = b * 128 + d
                nd = int(deg[p[pos]]) if pos < npc else 0
                masks[c, d, mdcol + nd:mdcol + D[b]] = -30000.0
            mdcol += int(D[b])
        # build idx images chunk by chunk (vectorized per chunk)
        for (b, j0, jc, ic, _mc) in chunks:
            ni = 128 * jc
            flat = np.zeros(ni, np.int16)
            for d in range(128):
                pos = b * 128 + d
                if pos >= npc:
                    continue
                g = p[pos]
                nd = int(deg[g])
                s = starts[g]
                lo = max(j0, 0)
                hi = min(j0 + jc, nd)
                for j in range(lo, hi):
                    flat[(j - j0) * 128 + d] = g2v[src_sorted[s + j]]
            img = flat.reshape(-1, 16).T  # [16, ni/16]
            for r in range(0, 128, 16):
                idx_imgs[c, r:r + 16, ic:ic + ni // 16] = img
    return dict(perms=perms, g2v=g2v, D=D, chunks=chunks, nblk=nblk,
                idx_imgs=idx_imgs, masks=masks, sumd=sumd, icols=icols_total)


def _blockdiag(a, f):
    """a: [H, C] -> [F=H*C? no: F x H] projection with A[h*C+c, h]=a[h,c]."""
    hh, cc = a.shape
    out = np.zeros((f, hh), np.float64)
    for h in range(hh):
        out[h * cc:(h + 1) * cc, h] = a[h]
    return out


def _prep_layers(inputs):
    """Fold weights host-side. Returns list of per-layer dicts."""
    layers = []
    specs = [("1", 8), ("2", 7), ("3", 7), ("4", 5)]
    for li, (s, H) in enumerate(specs):
        W = inputs["W" + s].astype(np.float64)
        a_s = inputs["as" + s].astype(np.float64)
        a_d = inputs["ad" + s].astype(np.float64)
        fin, fout = W.shape
        C = a_s.shape[1]
        F = fout if li < 3 else a_s.shape[0] * a_s.shape[1]  # same thing
        Was = W @ _blockdiag(a_s, fout)
        Wad = W @ _blockdiag(a_d, fout)
        Wext = np.concatenate([W, Was, Wad], axis=1)  # [fin, F+2H]
        d = dict(fin=fin, F=fout, H=H, C=C, Wext=Wext.astype(np.float16))
        if li < 3:
            g = inputs["g" + s].astype(np.float64)
            be = inputs["be" + s].astype(np.float64)
            m = inputs["m" + s].astype(np.float64)
            v = inputs["v" + s].astype(np.float64)
            b = inputs["b" + s].astype(np.float64)
            sc = g / np.sqrt(v + BN_EPS)
            bias = (b - m) * sc + be
            d["bnscale"] = sc.astype(np.float32)
            d["bnbias"] = bias.astype(np.float32)
        else:
            d["b4"] = inputs["b" + s].astype(np.float64)
        # table row width: F + H padded to multiple of 128 elems (256B fp16)
        d["R"] = ((fout + H + 127) // 128) * 128
        d["NW"] = fout + 2 * H
        layers.append(d)
    return layers


def _ktiles(fin):
    ks = []
    o = 0
    while o < fin:
        k = min(128, fin - o)
        ks.append((o, k))
        o += k
    return ks


def _ftiles(f):
    fs = []
    o = 0
    while o < f:
        w = min(128, f - o)
        fs.append((o, w))
        o += w
    return fs


# ------------------------------------------------------------- device build

def _build(n, npc, npad, layers, graph, pw):
    nblk = graph["nblk"]
    V = N_CORES * npad
    chunks = graph["chunks"]
    D = graph["D"]
    sumd = graph["sumd"]
    icols = graph["icols"]

    nc = bacc.Bacc("TRN2", target_bir_lowering=False, debug=False,
                   num_devices=N_CORES)

    # ---- IO
    xT_in = nc.dram_tensor("xT", [128, npad], FP16, kind="ExternalInput").ap()
    idx_in = nc.dram_tensor("idx", [128, icols], I16, kind="ExternalInput").ap()
    mask_in = nc.dram_tensor("mask", [128, sumd], FP16, kind="ExternalInput").ap()
    ident_in = nc.dram_tensor("ident", [128, 128], FP16, kind="ExternalInput").ap()
    ones_in = nc.dram_tensor("ones1", [1, 128], FP16, kind="ExternalInput").ap()
    b4_in = nc.dram_tensor("b4row", [1, layers[3]["NW"]], FP16, kind="ExternalInput").ap()
    w_in = []
    bn_in = []
    for li, L in enumerate(layers):
        wl = []
        for kt, (o, k) in enumerate(_ktiles(L["fin"])):
            wl.append(nc.dram_tensor(f"w{li}_{kt}", [k, L["NW"]], FP16,
                                     kind="ExternalInput").ap())
        w_in.append(wl)
        if li < 3:
            nft = len(_ftiles(L["F"]))
            bn_in.append((
                nc.dram_tensor(f"bnsc{li}", [128, nft], FP32, kind="ExternalInput").ap(),
                nc.dram_tensor(f"bnbi{li}", [128, nft], FP32, kind="ExternalInput").ap(),
            ))
        else:
            bn_in.append(None)
    out_dram = nc.dram_tensor("out", [npad, layers[3]["C"]], FP32, kind="ExternalOutput").ap()

    with tile.TileContext(nc) as tc:
        with tc.tile_pool(name="const", bufs=1) as cpool, \
             tc.tile_pool(name="work", bufs=3) as pool, \
             tc.tile_pool(name="zt", bufs=1) as zpool, \
             tc.tile_pool(name="psA", bufs=2, space="PSUM") as ppA, \
             tc.tile_pool(name="psB", bufs=2, space="PSUM") as ppB, \
             tc.tile_pool(name="dram", bufs=1, space="DRAM") as dpool:

            # ---- persistent SBUF
            ident = cpool.tile([128, 128], FP16)
            nc.sync.dma_start(ident[:], ident_in[:])
            ones1 = cpool.tile([1, 128], FP16)
            nc.sync.dma_start(ones1[:], ones_in[:])
            b4row = cpool.tile([1, layers[3]["NW"]], FP16)
            nc.sync.dma_start(b4row[:], b4_in[:])
            idx_sb = cpool.tile([128, icols], I16)
            nc.sync.dma_start(idx_sb[:], idx_in[:])
            mask_sb = cpool.tile([128, sumd], FP16)
            nc.sync.dma_start(mask_sb[:], mask_in[:])
            w_sb = []
            bn_sb = []
            for li, L in enumerate(layers):
                wl = []
                for kt, (o, k) in enumerate(_ktiles(L["fin"])):
                    t = cpool.tile([k, L["NW"]], FP16, tag=f"w{li}_{kt}")
                    nc.sync.dma_start(t[:], w_in[li][kt][:])
                    wl.append(t)
                w_sb.append(wl)
                if li < 3:
                    nft = len(_ftiles(L["F"]))
                    s = cpool.tile([128, nft], FP32, tag=f"bs{li}")
                    bbt = cpool.tile([128, nft], FP32, tag=f"bb{li}")
                    nc.sync.dma_start(s[:], bn_in[li][0][:])
                    nc.sync.dma_start(bbt[:], bn_in[li][1][:])
                    bn_sb.append((s, bbt))
                else:
                    bn_sb.append(None)
            ad_own = cpool.tile([128, nblk * 8], FP32)  # alpha_dst own nodes

            # zT for layer 0 = xT input
            zT = [zpool.tile([128, npad], FP16, tag="zt0_0", name="zt0_0")]
            nc.sync.dma_start(zT[0][:], xT_in[:])

            for li, L in enumerate(layers):
                F, H, C, R, NW = L["F"], L["H"], L["C"], L["R"], L["NW"]
                kts = _ktiles(L["fin"])
                fts = _ftiles(F)

                table = dpool.tile([V, R], FP16, tag=f"table{li}", addr_space="Shared")
                bounce = dpool.tile([npad, R], FP16, tag=f"bounce{li}")

                # ---------- phase A: project own nodes, build table via AG
                for mt in range(nblk):
                    psA = ppA.tile([128, 512], FP32, tag="gA")
                    n0 = min(NW, 512)
                    for kt, (o, k) in enumerate(kts):
                        nc.tensor.matmul(
                            psA[:, :n0],
                            zT[kt][:k, mt * 128:(mt + 1) * 128],
                            w_sb[li][kt][:, :n0],
                            start=(kt == 0),
                            stop=(kt == len(kts) - 1 and li != 3),
                        )
                    if NW > 512:
                        psB = ppB.tile([128, 128], FP32, tag="gB")
                        for kt, (o, k) in enumerate(kts):
                            nc.tensor.matmul(
                                psB[:, :NW - 512],
                                zT[kt][:k, mt * 128:(mt + 1) * 128],
                                w_sb[li][kt][:, 512:NW],
                                start=(kt == 0),
                                stop=(kt == len(kts) - 1),
                            )
                    if li == 3:
                        nc.tensor.matmul(psA[:, :NW], ones1[:], b4row[:],
                                         start=False, stop=True)
                    own = pool.tile([128, R], FP16, tag="own")
                    if R > F + H:
                        nc.vector.memset(own[:, F + H:R], 0.0)
                    c0 = min(F + H, 512)
                    nc.scalar.copy(own[:, :c0], psA[:, :c0])
                    if F + H > 512:
                        nc.scalar.copy(own[:, 512:F + H], psB[:, :F + H - 512])
                    # alpha_dst -> resident fp32
                    if F + 2 * H <= 512:
                        nc.scalar.copy(ad_own[:, mt * 8:mt * 8 + H],
                                       psA[:, F + H:F + 2 * H])
                    else:
                        nc.scalar.copy(ad_own[:, mt * 8:mt * 8 + H],
                                       psB[:, F + H - 512:F + 2 * H - 512])
                    nc.sync.dma_start(bounce[mt * 128:(mt + 1) * 128, :],
                                      own[:])
                nc.gpsimd.collective_compute(
                    "AllGather", mybir.AluOpType.bypass,
                    replica_groups=[list(range(N_CORES))],
                    ins=[bounce[:].opt()], outs=[table[:].opt()],
                )

                # ---------- phase B: edge aggregation per block
                zT_next = None
                if li < 3:
                    zT_next = [zpool.tile([128, npad], FP16,
                                           tag=f"zt{li + 1}_{t}",
                                           name=f"zt{li + 1}_{t}")
                               for t in range(len(fts))]
                for b in range(nblk):
                    psagg = ppA.tile([128, 512], FP32, tag="agg")
                    p_all = pool.tile([128, int(D[b]), 8], FP16, tag="pall")
                    for (cb, j0, jc, ic, mc) in chunks:
                        if cb != b:
                            continue
                        ni = 128 * jc
                        G = pool.tile([128, jc, R], FP16, tag="G")
                        nc.gpsimd.dma_gather(
                            out_ap=G[:],
                            in_ap=table[:],
                            idxs_ap=idx_sb[:, ic:ic + ni // 16],
                            num_idxs=ni,
                            num_idxs_reg=ni,
                            elem_size=R,
                        )
                        gs = pool.tile([128, jc, H], FP32, tag="gs")
                        nc.vector.tensor_tensor(
                            out=gs[:],
                            in0=G[:, :, F:F + H],
                            in1=ad_own[:, b * 8:b * 8 + H][:, None, :]
                                .broadcast_to([128, jc, H]),
                            op=mybir.AluOpType.add,
                        )
                        nc.vector.tensor_tensor(
                            out=gs[:],
                            in0=gs[:],
                            in1=mask_sb[:, mc:mc + jc][:, :, None]
                                .broadcast_to([128, jc, H]),
                            op=mybir.AluOpType.add,
                        )
                        nc.vector.scalar_tensor_tensor(
                            out=gs[:], in0=gs[:], scalar=NEG_SLOPE, in1=gs[:],
                            op0=mybir.AluOpType.mult, op1=mybir.AluOpType.max,
                        )
                        nc.scalar.activation(p_all[:, j0:j0 + jc, :H], gs[:],
                                             mybir.ActivationFunctionType.Exp)
                        M = pool.tile([128, jc, F], FP16, tag="M")
                        nc.vector.tensor_tensor(
                            out=M[:].rearrange("p j (h c) -> p j h c", h=H),
                            in0=G[:, :, :F].rearrange("p j (h c) -> p j h c", h=H),
                            in1=p_all[:, j0:j0 + jc, :H][:, :, :, None]
                                .broadcast_to([128, jc, H, C]),
                            op=mybir.AluOpType.mult,
                        )
                        for j in range(jc):
                            nc.tensor.matmul(
                                psagg[:, :F], ident[:], M[:, j, :],
                                start=(j0 + j == 0),
                                stop=(j0 + j == int(D[b]) - 1),
                            )
                    denom = pool.tile([128, H], FP32, tag="denom")
                    nc.vector.tensor_reduce(
                        denom[:], p_all[:, :, :H].rearrange("p j h -> p h j"),
                        axis=mybir.AxisListType.X, op=mybir.AluOpType.add)
                    nc.vector.tensor_scalar(denom[:], denom[:], 1e-16, None,
                                            op0=mybir.AluOpType.add)
                    rden = pool.tile([128, H], FP32, tag="rden")
                    nc.vector.reciprocal(rden[:], denom[:])
                    if li == 3:
                        nc.vector.tensor_scalar(rden[:], rden[:], 1.0 / H, None,
                                                op0=mybir.AluOpType.mult)
                        out5 = pool.tile([128, H, C], FP32, tag="out5")
                        nc.vector.tensor_tensor(
                            out=out5[:],
                            in0=psagg[:, :F].rearrange("p (h c) -> p h c", h=H),
                            in1=rden[:, :, None].broadcast_to([128, H, C]),
                            op=mybir.AluOpType.mult,
                        )
                        mean = pool.tile([128, C], FP32, tag="mean")
                        nc.vector.tensor_reduce(
                            mean[:], out5[:].rearrange("p h c -> p c h"),
                            axis=mybir.AxisListType.X, op=mybir.AluOpType.add)
                        mx = pool.tile([128, 1], FP32, tag="mx")
                        nc.vector.tensor_reduce(mx[:], mean[:],
                                                axis=mybir.AxisListType.X,
                                                op=mybir.AluOpType.max)
                        negm = pool.tile([128, 1], FP32, tag="negm")
                        nc.vector.tensor_scalar(negm[:], mx[:], -1.0, None,
                                                op0=mybir.AluOpType.mult)
                        esc = pool.tile([128, C], FP32, tag="esc")
                        esum = pool.tile([128, 1], FP32, tag="esum")
                        nc.scalar.activation(esc[:], mean[:],
                                             mybir.ActivationFunctionType.Exp,
                                             bias=negm[:], accum_out=esum[:])
                        lns = pool.tile([128, 1], FP32, tag="lns")
                        nc.scalar.activation(lns[:], esum[:],
                                             mybir.ActivationFunctionType.Ln)
                        off = pool.tile([128, 1], FP32, tag="off")
                        nc.vector.tensor_tensor(out=off[:], in0=negm[:],
                                                in1=lns[:],
                                                op=mybir.AluOpType.subtract)
                        fin40 = pool.tile([128, C], FP32, tag="fin40")
                        nc.vector.tensor_scalar(fin40[:], mean[:], off[:], None,
                                                op0=mybir.AluOpType.add)
                        nc.sync.dma_start(out_dram[b * 128:(b + 1) * 128, :],
                                          fin40[:])
                    else:
                        zb = pool.tile([128, F], FP16, tag="zb")
                        nc.vector.tensor_tensor(
                            out=zb[:].rearrange("p (h c) -> p h c", h=H),
                            in0=psagg[:, :F].rearrange("p (h c) -> p h c", h=H),
                            in1=rden[:, :, None].broadcast_to([128, H, C]),
                            op=mybir.AluOpType.mult,
                        )
                        for ft, (fo, fw) in enumerate(fts):
                            psT = ppB.tile([128, 128], FP16, tag="tr")
                            nc.tensor.transpose(psT[:fw, :], zb[:, fo:fo + fw],
                                                ident[:])
                            t32 = pool.tile([128, 128], FP32, tag="t32")
                            nc.scalar.activation(
                                t32[:fw, :], psT[:fw, :],
                                mybir.ActivationFunctionType.Identity,
                                bias=bn_sb[li][1][:fw, ft:ft + 1],
                                scale=bn_sb[li][0][:fw, ft:ft + 1],
                            )
                            nc.vector.scalar_tensor_tensor(
                                out=zT_next[ft][:fw, b * 128:(b + 1) * 128],
                                in0=t32[:fw, :], scalar=float(pw),
                                in1=t32[:fw, :],
                                op0=mybir.AluOpType.mult,
                                op1=mybir.AluOpType.max,
                            )
                if li < 3:
                    zT = zT_next
    nc.compile()
    return nc


# ------------------------------------------------------------------- kernel

def _np_reference(inputs):
    """Plain numpy port of the jax reference (for testing)."""
    x = inputs["x"].astype(np.float64)
    n = x.shape[0]
    e = inputs["edge_index"]
    src = np.concatenate([e[0], np.arange(n)]).astype(np.int64)
    dst = np.concatenate([e[1], np.arange(n)]).astype(np.int64)
    pw = float(np.asarray(inputs["pw"]).reshape(-1)[0])

    def gat(h_in, s, heads, out_c, concat):
        W = inputs["W" + s].astype(np.float64)
        a_s = inputs["as" + s].astype(np.float64)
        a_d = inputs["ad" + s].astype(np.float64)
        b = inputs["b" + s].astype(np.float64)
        h = (h_in @ W).reshape(n, heads, out_c)
        als = np.einsum("nhc,hc->nh", h, a_s)
        ald = np.einsum("nhc,hc->nh", h, a_d)
        ee = als[src] + ald[dst]
        ee = np.where(ee > 0, ee, NEG_SLOPE * ee)
        m = np.full((n, heads), -np.inf)
        np.maximum.at(m, dst, ee)
        m = np.where(np.isfinite(m), m, 0.0)
        p = np.exp(ee - m[dst])
        den = np.zeros((n, heads))
        np.add.at(den, dst, p)
        alpha = p / (den[dst] + 1e-16)
        out = np.zeros((n, heads, out_c))
        np.add.at(out, dst, alpha[:, :, None] * h[src])
        out = out.reshape(n, heads * out_c) if concat else out.mean(1)
        return out + b

    def bn(h, s):
        g = inputs["g" + s].astype(np.float64)
        be = inputs["be" + s].astype(np.float64)
        m = inputs["m" + s].astype(np.float64)
        v = inputs["v" + s].astype(np.float64)
        return (h - m) * (g / np.sqrt(v + BN_EPS)) + be

    prelu = lambda h: np.where(h > 0, h, pw * h)
    C = inputs["as1"].shape[1]
    h = gat(x, "1", 8, C, True)
    h = prelu(bn(h, "1"))
    h = gat(h, "2", 7, C, True)
    h = prelu(bn(h, "2"))
    h = gat(h, "3", 7, C, True)
    h = prelu(bn(h, "3"))
    h = gat(h, "4", 5, inputs["as4"].shape[1], False)
    h = h - h.max(1, keepdims=True)
    lse = np.log(np.exp(h).sum(1, keepdims=True))
    return (h - lse).astype(np.float32)


def _make_in_maps(inputs, graph, layers, npc, npad):
    x = inputs["x"]
    shared = {
        "ident": np.eye(128, dtype=np.float16),
        "ones1": np.ones((1, 128), np.float16),
    }
    b4r = np.zeros((1, layers[3]["NW"]), np.float16)
    H4, C4 = layers[3]["H"], layers[3]["C"]
    b4r[0, :H4 * C4] = np.tile(layers[3]["b4"], H4).astype(np.float16)
    shared["b4row"] = b4r
    for li, L in enumerate(layers):
        for kt, (o, k) in enumerate(_ktiles(L["fin"])):
            shared[f"w{li}_{kt}"] = np.ascontiguousarray(
                L["Wext"][o:o + k, :]).astype(np.float16)
        if li < 3:
            nft = len(_ftiles(L["F"]))
            sc = np.zeros((128, nft), np.float32)
            bi = np.zeros((128, nft), np.float32)
            for ft, (fo, fw) in enumerate(_ftiles(L["F"])):
                sc[:fw, ft] = L["bnscale"][fo:fo + fw]
                bi[:fw, ft] = L["bnbias"][fo:fo + fw]
            shared[f"bnsc{li}"] = sc
            shared[f"bnbi{li}"] = bi

    in_maps = []
    for c in range(N_CORES):
        xT = np.zeros((128, npad), np.float16)
        xT[:, :npc] = x[graph["perms"][c]].T.astype(np.float16)
        m = dict(shared)
        m["xT"] = xT
        m["idx"] = np.ascontiguousarray(graph["idx_imgs"][c])
        m["mask"] = np.ascontiguousarray(graph["masks"][c])
        in_maps.append(m)
    return in_maps


def kernel(_sim=False, **inputs):
    x = inputs["x"]
    edge_index = inputs["edge_index"]
    n = x.shape[0]
    npc = n // N_CORES
    npad = ((npc + 127) // 128) * 128

    graph = _prep_graph(n, edge_index, npc, npad)
    layers = _prep_layers(inputs)
    pw = float(np.asarray(inputs["pw"]).reshape(-1)[0])

    nc = _build(n, npc, npad, layers, graph, pw)
    in_maps = _make_in_maps(inputs, graph, layers, npc, npad)

    if _sim:
        from concourse.bass_interp import MultiCoreSim
        sim = MultiCoreSim(nc, num_cores=N_CORES, trace=False,
                           require_finite=True, require_nnan=True)
        cores = list(sim.cores.values())
        for c in range(N_CORES):
            for name, arr in in_maps[c].items():
                cores[c].tensor(name)[:] = arr
        sim.simulate(check_with_hw=False)
        results = [{"out": np.array(cores[c].tensor("out"))}
                   for c in range(N_CORES)]
    else:
        res = bass_utils.run_bass_kernel_spmd(
            nc, in_maps, core_ids=list(range(N_CORES)))
        results = res.results

    ncls = results[0]["out"].shape[1]
    out = np.empty((n, ncls), np.float32)
    for c in range(N_CORES):
        out[graph["perms"][c]] = results[c]["out"][:npc]
    return out


# ----------------------------------------------------------- timing support

def _make_runner(nc):
    """Reusable jitted PJRT executor for the prebuilt Bass module (axon)."""
    import jax
    import numpy as _np
    from jax.experimental.shard_map import shard_map
    from jax.sharding import Mesh, NamedSharding, PartitionSpec
    from concourse import bass2jax
    import concourse.mybir as mb

    bass2jax.install_neuronx_cc_hook()
    partition_name = (nc.partition_id_tensor.name
                      if nc.partition_id_tensor else None)
    in_names, out_names, out_avals, zero_outs = [], [], [], []
    for alloc in nc.m.functions[0].allocations:
        if not isinstance(alloc, mb.MemoryLocationSet):
            continue
        name = alloc.memorylocations[0].name
        if alloc.kind == "ExternalInput":
            if name != partition_name:
                in_names.append(name)
        elif alloc.kind == "ExternalOutput":
            shape = tuple(alloc.tensor_shape)
            dtype = mb.dt.np(alloc.dtype)
            out_names.append(name)
            out_avals.append(jax.core.ShapedArray(shape, dtype))
            zero_outs.append(_np.zeros(shape, dtype))
    n_params = len(in_names)
    all_names = in_names + out_names
    if partition_name is not None:
        all_names.append(partition_name)

    def _body(*args):
        operands = list(args)
        if partition_name is not None:
            operands.append(bass2jax.partition_id_tensor())
        outs = bass2jax._bass_exec_p.bind(
            *operands,
            out_avals=tuple(out_avals),
            in_names=tuple(all_names),
            out_names=tuple(out_names),
            lowering_input_output_aliases=(),
            sim_require_finite=True,
            sim_require_nnan=True,
            nc=nc,
        )
        return tuple(outs)

    devices = jax.devices()[:N_CORES]
    mesh = Mesh(np.asarray(devices), ("core",))
    n_outs = len(out_names)
    donate = tuple(range(n_params, n_params + n_outs))
    sharded = jax.jit(
        shard_map(_body, mesh=mesh,
                  in_specs=(PartitionSpec("core"),) * (n_params + n_outs),
                  out_specs=(PartitionSpec("core"),) * n_outs,
                  check_rep=False),
        donate_argnums=donate, keep_unused=True)
    shard = NamedSharding(mesh, PartitionSpec("core"))
    return sharded, in_names, out_names, out_avals, zero_outs, shard


def time_kernel(n_iters=6, **inputs):
    """Best-of wall time of the device execution (ns)."""
    import time as _time
    import jax

    x = inputs["x"]
    n = x.shape[0]
    npc = n // N_CORES
    npad = ((npc + 127) // 128) * 128
    graph = _prep_graph(n, inputs["edge_index"], npc, npad)
    layers = _prep_layers(inputs)
    pw = float(np.asarray(inputs["pw"]).reshape(-1)[0])
    nc = _build(n, npc, npad, layers, graph, pw)
    in_maps = _make_in_maps(inputs, graph, layers, npc, npad)

    fn, in_names, out_names, out_avals, zero_outs, shard = _make_runner(nc)
    concat_in = [jax.device_put(
        np.concatenate([np.asarray(in_maps[c][nm]) for c in range(N_CORES)], axis=0),
        shard) for nm in in_names]

    def zeros():
        return [jax.device_put(
            np.zeros((N_CORES * z.shape[0], *z.shape[1:]), z.dtype), shard)
            for z in zero_outs]

    outs = fn(*concat_in, *zeros())   # compile + warm-up
    jax.block_until_ready(outs)
    best = float("inf")
    for _ in range(n_iters):
        zs = zeros()
        jax.block_until_ready(zs)
        t0 = _time.perf_counter()
        outs = fn(*concat_in, *zs)
        jax.block_until_ready(outs)
        best = min(best, _time.perf_counter() - t0)
    return best * 1e9
